# revision 1
# baseline (speedup 1.0000x reference)
"""Trainium2 Bass kernel for nn_DecoderTransformer segment_reduce problem.

Computes, per batch sample b (one NeuronCore each, 8 cores total):
    sums[s, :]   = sum over (n, k) with indexes[b, n, k] == s of graph_output[b, n, :]
    counts[s]    = multiplicity of s in indexes[b]
    graph_hidden = (sums + 1e-8) / max(counts, 1)
    enc[b]       = concat([graph_hidden, seq_output[b]], axis=-1)   # [2048, 1024]
Returns (enc [8, 2048, 1024] f32, hidden [8, 1024] f32 passthrough).

Device algorithm (per core):
  Host sorts the 2048 (n, k) updates by target s. The sorted stream is cut in
  16 chunks of 128 updates. For each chunk: indirect-DMA gather of its source
  rows of graph_output (128 x 512 f32), and for each 128-row output tile its
  targets touch, a selection matrix Sel[j, m] = (t_j == 128*tile + m) built on
  the vector engine with one is_equal tensor_scalar against an iota row. The
  scatter-add is then Sel.T @ gathered_rows on the tensor engine, accumulated
  in PSUM across chunks (targets split across chunk boundaries sum correctly
  via PSUM accumulation). The PSUM->SBUF pass fuses (sums + eps) * r with
  r = 1/max(counts, 1) host-precomputed from the index metadata. seq_output is
  staged into the right half of a [128, 1024] tile so each output tile is one
  contiguous 512 KiB DMA.
"""

import numpy as np

import concourse.bass as bass
import concourse.bacc as bacc
import concourse.tile as tile
from concourse import mybir
from concourse.bass_utils import run_bass_kernel_spmd

B, S, N, K = 8, 2048, 512, 4
DG, DSEQ, H = 512, 512, 1024
P = 128
N_CHUNKS = (N * K) // P  # 16
N_TILES = S // P  # 16
EPS = 1e-8

# Filled by kernel() on every call; read by test harnesses.
LAST_EXEC_NS = None
LAST_RESULTS = None


def _host_metadata(indexes):
    """Per-core sorted-update metadata + the SPMD-uniform (chunk, tile) pairs."""
    per_core = []
    for b in range(B):
        t_flat = np.asarray(indexes[b], dtype=np.int64).reshape(-1)  # (n, k) order
        order = np.argsort(t_flat, kind="stable")
        t_sorted = t_flat[order]
        src = (order // K).astype(np.int32)
        counts = np.bincount(t_flat, minlength=S)
        r = (1.0 / np.maximum(counts, 1)).astype(np.float32)
        per_core.append((t_sorted, src, r))

    # Union coverage: chunk c touches output tiles [lo[c], hi[c]] across cores.
    lo = np.full(N_CHUNKS, N_TILES, np.int64)
    hi = np.full(N_CHUNKS, -1, np.int64)
    for t_sorted, _, _ in per_core:
        tc_lo = t_sorted.reshape(N_CHUNKS, P)[:, 0] // P
        tc_hi = t_sorted.reshape(N_CHUNKS, P)[:, -1] // P
        lo = np.minimum(lo, tc_lo)
        hi = np.maximum(hi, tc_hi)

    # pair list in (chunk, tile) order; per tile, list of (pair_idx, chunk)
    pairs = []  # (c, tau)
    for c in range(N_CHUNKS):
        for tau in range(int(lo[c]), int(hi[c]) + 1):
            pairs.append((c, tau))
    tile_chunks = {tau: [] for tau in range(N_TILES)}
    for pi, (c, tau) in enumerate(pairs):
        tile_chunks[tau].append((pi, c))

    n_pairs = len(pairs)
    src_cols = np.zeros((B, P, N_CHUNKS), np.int32)
    shift_cols = np.zeros((B, P, n_pairs), np.float32)
    r_cols = np.zeros((B, P, N_TILES), np.float32)
    for b in range(B):
        t_sorted, src, r = per_core[b]
        src_cols[b] = src.reshape(N_CHUNKS, P).T
        r_cols[b] = r.reshape(N_TILES, P).T
        ts_chunks = t_sorted.reshape(N_CHUNKS, P)
        for pi, (c, tau) in enumerate(pairs):
            shift_cols[b, :, pi] = (ts_chunks[c] - P * tau).astype(np.float32)
    return pairs, tile_chunks, src_cols, shift_cols, r_cols


def _build_kernel(pairs, tile_chunks):
    n_pairs = len(pairs)
    f32 = mybir.dt.float32
    nc = bacc.Bacc("TRN2", target_bir_lowering=False, debug=False)
    seq = nc.dram_tensor("seq", [S, DSEQ], f32, kind="ExternalInput")
    g = nc.dram_tensor("g", [N, DG], f32, kind="ExternalInput")
    srcm = nc.dram_tensor("srcm", [P, N_CHUNKS], mybir.dt.int32, kind="ExternalInput")
    shiftm = nc.dram_tensor("shiftm", [P, n_pairs], f32, kind="ExternalInput")
    rm = nc.dram_tensor("rm", [P, N_TILES], f32, kind="ExternalInput")
    enc = nc.dram_tensor("enc", [S, DG + DSEQ], f32, kind="ExternalOutput")

    with tile.TileContext(nc) as tc:
        with (
            tc.tile_pool(name="const", bufs=1) as const,
            tc.tile_pool(name="gather", bufs=N_CHUNKS) as gather_pool,
            tc.tile_pool(name="eq", bufs=n_pairs) as eq_pool,
            tc.tile_pool(name="out", bufs=4) as out_pool,
            tc.tile_pool(name="psum", bufs=8, space="PSUM") as psum_pool,
        ):
            iota_i = const.tile([P, P], mybir.dt.int32)
            nc.gpsimd.iota(iota_i[:], pattern=[[1, P]], base=0, channel_multiplier=0)
            iota_f = const.tile([P, P], f32)
            nc.vector.tensor_copy(out=iota_f[:], in_=iota_i[:])

            src_sb = const.tile([P, N_CHUNKS], mybir.dt.int32)
            nc.sync.dma_start(out=src_sb[:], in_=srcm[:, :])
            shift_sb = const.tile([P, n_pairs], f32)
            nc.sync.dma_start(out=shift_sb[:], in_=shiftm[:, :])
            r_sb = const.tile([P, N_TILES], f32)
            nc.sync.dma_start(out=r_sb[:], in_=rm[:, :])
            epsr_sb = const.tile([P, N_TILES], f32)
            nc.vector.tensor_scalar(
                out=epsr_sb[:], in0=r_sb[:], scalar1=EPS, scalar2=None,
                op0=mybir.AluOpType.mult,
            )

            # Per-chunk gathers and per-pair selection matrices.
            gathers = []
            for c in range(N_CHUNKS):
                gt = gather_pool.tile([P, DG], f32)
                nc.gpsimd.indirect_dma_start(
                    out=gt[:], out_offset=None, in_=g[:, :],
                    in_offset=bass.IndirectOffsetOnAxis(ap=src_sb[:, c:c + 1], axis=0),
                )
                gathers.append(gt)
            eqs = []
            for pi in range(n_pairs):
                eqt = eq_pool.tile([P, P], f32)
                nc.vector.tensor_scalar(
                    out=eqt[:], in0=iota_f[:], scalar1=shift_sb[:, pi:pi + 1],
                    scalar2=None, op0=mybir.AluOpType.is_equal,
                )
                eqs.append(eqt)

            # Per output tile: matmul-accumulate, fused epilogue, concat, store.
            for tau in range(N_TILES):
                ot = out_pool.tile([P, DG + DSEQ], f32)
                nc.sync.dma_start(
                    out=ot[:, DG:], in_=seq[tau * P:(tau + 1) * P, :]
                )
                plist = tile_chunks[tau]
                if plist:
                    ps = psum_pool.tile([P, DG], f32, space="PSUM")
                    for i, (pi, c) in enumerate(plist):
                        nc.tensor.matmul(
                            out=ps[:], lhsT=eqs[pi][:], rhs=gathers[c][:],
                            start=(i == 0), stop=(i == len(plist) - 1),
                        )
                    if tau % 2 == 0:
                        nc.vector.tensor_scalar(
                            out=ot[:, :DG], in0=ps[:], scalar1=EPS,
                            scalar2=r_sb[:, tau:tau + 1],
                            op0=mybir.AluOpType.add, op1=mybir.AluOpType.mult,
                        )
                    else:
                        nc.scalar.activation(
                            out=ot[:, :DG], in_=ps[:],
                            func=mybir.ActivationFunctionType.Identity,
                            bias=epsr_sb[:, tau:tau + 1],
                            scale=r_sb[:, tau:tau + 1],
                        )
                else:
                    nc.vector.memset(ot[:, :DG], EPS)
                nc.sync.dma_start(
                    out=enc[tau * P:(tau + 1) * P, :], in_=ot[:]
                )
    nc.compile()
    return nc


def kernel(seq_output, graph_output, hidden, indexes, _trace=False):
    global LAST_EXEC_NS, LAST_RESULTS
    seq_output = np.ascontiguousarray(np.asarray(seq_output, dtype=np.float32))
    graph_output = np.ascontiguousarray(np.asarray(graph_output, dtype=np.float32))
    hidden_np = np.asarray(hidden)

    pairs, tile_chunks, src_cols, shift_cols, r_cols = _host_metadata(indexes)
    nc = _build_kernel(pairs, tile_chunks)

    in_maps = [
        {
            "seq": seq_output[b],
            "g": graph_output[b],
            "srcm": np.ascontiguousarray(src_cols[b]),
            "shiftm": np.ascontiguousarray(shift_cols[b]),
            "rm": np.ascontiguousarray(r_cols[b]),
        }
        for b in range(B)
    ]
    res = run_bass_kernel_spmd(nc, in_maps, core_ids=list(range(B)), trace=_trace)
    LAST_EXEC_NS = res.exec_time_ns
    LAST_RESULTS = res
    enc = np.stack([res.results[b]["enc"] for b in range(B)], axis=0)
    hidden_flat = np.ascontiguousarray(hidden_np.reshape(hidden_np.shape[0], -1))
    return enc, hidden_flat
